# revision 15
# baseline (speedup 1.0000x reference)
"""Self-contained Trainium2 Bass kernel for nn_BestEllipseLoss_5720896438361.

kernel(output, target): full [512,128,128] f32 inputs -> scalar f32 loss.
Shards batch across 8 NeuronCores (64 samples each), one SPMD Bass kernel.

v3: hybrid metric evaluation — levels {0,1} via gpsimd prefix-sum gathers,
levels {2,3,4} via VE interval masks + PE fp32 column-sum matmuls (the
gpsimd ap_gather costs ~26ns/index, so splitting the 82K gathered values
between engines balances the machine). Output side computed at all 5
levels during the gather window; select-at-best after argmax. minmaxO on
gpsimd. G=8 moment matmul batching.
"""
import sys
if "/opt/trn_rl_repo" not in sys.path:
    sys.path.insert(0, "/opt/trn_rl_repo")

import numpy as np

import concourse.bass as bass
import concourse.bacc as bacc
import concourse.tile as tile
import concourse.mybir as mybir
import concourse.bass_isa as bass_isa

F32 = mybir.dt.float32
BF16 = mybir.dt.bfloat16
I32 = mybir.dt.int32
I16 = mybir.dt.int16
Alu = mybir.AluOpType
Act = mybir.ActivationFunctionType
AX = mybir.AxisListType

EPS = np.float32(1e-8)
LEVELS = [np.float32(0.3), np.float32(0.4), np.float32(0.5), np.float32(0.6), np.float32(0.7)]
NL = 5
NLG = 2          # gather levels (0..NLG-1); mask levels NLG..NL-1
H = 128
W = 128
NPIX = float(H * W)
NG = 16          # moment x-groups (8 positions each)
GP = 8           # x positions per group
KJ = 16          # basis cols per position (6 hi + 6 lo + 4 zero pad -> 128-col lhsT enables FWL)

_x = np.arange(W, dtype=np.float64)
_y = np.arange(H, dtype=np.float64)
# basis order j: {1, y, y^2, x, x*y, x^2}
C_B = np.array([
    H * W, W * _y.sum(), W * (_y ** 2).sum(),
    H * _x.sum(), _x.sum() * _y.sum(), H * (_x ** 2).sum(),
], dtype=np.float64).astype(np.float32)


def emit(nc, tc, NS=64, debug=False):
    F = NS * W
    SL = NL * NS          # l-major (l*NS + s)
    GSL = NLG * NS        # gather-level columns
    MSL = (NL - NLG) * NS  # mask-level columns
    NCH = 8               # mask chunks
    SCH = NS // NCH       # samples per chunk

    dbg = {}
    def DBG(name, ap):
        if not debug:
            return
        t = nc.dram_tensor(f"dbg_{name}", list(ap.shape), ap.dtype, kind="ExternalOutput")
        nc.sync.dma_start(t[...], ap)
        dbg[name] = t

    t_in = nc.dram_tensor("t", [NS, H, W], F32, kind="ExternalInput")
    o_in = nc.dram_tensor("o", [NS, H, W], F32, kind="ExternalInput")
    loss_out = nc.dram_tensor("loss", [NS, 1], F32, kind="ExternalOutput")

    with tc.tile_pool(name="big", bufs=1) as big, \
         tc.tile_pool(name="med", bufs=1) as med, \
         tc.tile_pool(name="sml", bufs=1) as sml, \
         tc.tile_pool(name="fld", bufs=1) as fld, \
         tc.tile_pool(name="ps", bufs=2, space="PSUM") as ps, \
         tc.tile_pool(name="ps1", bufs=1, space="PSUM") as ps1:

        # ================= constants =================
        yi = sml.tile([128, 1], I32)
        nc.gpsimd.iota(yi[:], pattern=[[0, 1]], base=0, channel_multiplier=1)
        yv = sml.tile([128, 1], F32)
        nc.vector.tensor_copy(yv[:], yi[:])
        y2v = sml.tile([128, 1], F32)
        nc.vector.tensor_tensor(out=y2v[:], in0=yv[:], in1=yv[:], op=Alu.mult)

        ei = med.tile([128, 128], I32, tag="scrA")
        nc.gpsimd.iota(ei[:], pattern=[[1, 128]], base=0, channel_multiplier=-1)
        eif = med.tile([128, 128], F32, tag="scrB")
        nc.vector.tensor_copy(eif[:], ei[:])
        eye128 = med.tile([128, 128], F32)
        nc.vector.tensor_scalar(eye128[:], eif[:], 0.0, None, Alu.is_equal)

        e16i = sml.tile([128, 16], I32)
        nc.gpsimd.iota(e16i[:], pattern=[[1, 16]], base=0, channel_multiplier=-1)
        e16s = sml.tile([128, 16], F32)
        nc.vector.tensor_copy(e16s[:], e16i[:])
        nc.vector.tensor_scalar(e16s[:], e16s[:], 1.0 / 16.0, None, Alu.mult)
        e16t = sml.tile([128, 16], I32)
        nc.vector.tensor_copy(e16t[:], e16s[:])
        e16tf = sml.tile([128, 16], F32)
        nc.vector.tensor_copy(e16tf[:], e16t[:])
        eye16 = sml.tile([128, 16], F32)
        nc.vector.tensor_tensor(out=eye16[:], in0=e16s[:], in1=e16tf[:], op=Alu.is_equal)
        eyeneg16 = sml.tile([128, 16], F32)
        nc.vector.tensor_scalar(eyeneg16[:], eye16[:], -1.0, None, Alu.mult)

        onescol = sml.tile([128, 1], F32)
        nc.gpsimd.memset(onescol[:], 1.0)

        lvl_bias = []
        for l in range(NL):
            b = sml.tile([128, 1], F32, name=f"lvlb{l}")
            nc.gpsimd.memset(b[:], -float(LEVELS[l] - np.float32(0.5)))
            lvl_bias.append(b)

        # SEL_t [96, 6] fold matrices: SEL_t[q, m] = (q == t*12+m) or (q == t*12+6+m)
        di = med.tile([128, 6], I32, tag="scrC")
        nc.gpsimd.iota(di[:], pattern=[[-1, 6]], base=0, channel_multiplier=1)
        df = med.tile([128, 6], F32, tag="scrD")
        nc.vector.tensor_copy(df[:], di[:])
        SELS = []
        for t in range(GP):
            s1 = med.tile([128, 6], F32, name=f"sel{t}")
            nc.vector.tensor_scalar(s1[:], df[:], float(KJ * t), None, Alu.is_equal)
            s2 = med.tile([128, 6], F32, name=f"sel2_{t}", tag="scrE")
            nc.vector.tensor_scalar(s2[:], df[:], float(KJ * t + 6), None, Alu.is_equal)
            nc.vector.tensor_tensor(out=s1[:], in0=s1[:], in1=s2[:], op=Alu.add)
            SELS.append(s1)

        # ---- moment lhsT table [128, NG*96] bf16 (hi/lo split fp32 basis) ----
        mast = med.tile([128, 768], F32, tag="scrA")
        nc.gpsimd.memset(mast[:], 1.0)
        xri = sml.tile([1, W], I32)
        nc.gpsimd.iota(xri[:], pattern=[[1, W]], base=0, channel_multiplier=0)
        xrf = sml.tile([1, W], F32)
        nc.vector.tensor_copy(xrf[:], xri[:])
        x2rf = sml.tile([1, W], F32)
        nc.vector.tensor_tensor(out=x2rf[:], in0=xrf[:], in1=xrf[:], op=Alu.mult)
        xfull = med.tile([128, W], F32, tag="scrB")
        nc.gpsimd.partition_broadcast(xfull[:], xrf[:], channels=128)
        x2full = med.tile([128, W], F32, tag="scrF")
        nc.gpsimd.partition_broadcast(x2full[:], x2rf[:], channels=128)
        # bf16 x pattern [128, SCH*W] (unit stride) for mask compares
        xsx = med.tile([128, SCH * W], BF16, name="xsx", tag="xsx2")
        nc.vector.tensor_copy(xsx[:].rearrange("p (s x) -> p s x", s=SCH),
                              xfull[:].rearrange("p (a x) -> p a x", a=1).to_broadcast((128, SCH, W)))
        mv = mast[:].rearrange("p (g t j) -> p g t j", g=NG, t=GP)
        xfv = xfull[:].rearrange("p (g t) -> p g t", g=NG)
        x2fv = x2full[:].rearrange("p (g t) -> p g t", g=NG)
        nc.vector.tensor_copy(mv[:, :, :, 3:4], xfv.to_broadcast((128, NG, GP, 1)))
        nc.vector.tensor_copy(mv[:, :, :, 4:5], xfv.to_broadcast((128, NG, GP, 1)))
        nc.vector.tensor_copy(mv[:, :, :, 5:6], x2fv.to_broadcast((128, NG, GP, 1)))
        mgt = mast[:].rearrange("p (gt j) -> p gt j", j=6)
        nc.vector.tensor_scalar(mgt[:, :, 1:2], mgt[:, :, 1:2], yv[:], None, Alu.mult)
        nc.vector.tensor_scalar(mgt[:, :, 4:5], mgt[:, :, 4:5], yv[:], None, Alu.mult)
        nc.vector.tensor_scalar(mgt[:, :, 2:3], mgt[:, :, 2:3], y2v[:], None, Alu.mult)
        hi24 = med.tile([128, 768], BF16, tag="scrG")
        nc.vector.tensor_copy(hi24[:], mast[:])
        table = med.tile([128, NG * 128], BF16)
        nc.gpsimd.memset(table[:], 0.0)
        tvv = table[:].rearrange("p (g t j) -> p g t j", g=NG, t=GP)
        nc.vector.tensor_copy(tvv[:, :, :, 0:6], hi24[:].rearrange("p (g t j) -> p g t j", g=NG, t=GP))
        nc.vector.tensor_tensor(out=tvv[:, :, :, 6:12],
                                in0=mast[:].rearrange("p (g t j) -> p g t j", g=NG, t=GP),
                                in1=hi24[:].rearrange("p (g t j) -> p g t j", g=NG, t=GP),
                                op=Alu.subtract)

        # sbase [128, GSL]: value at (l, s) = W*s   (gather levels only)
        sbi = med.tile([128, GSL], I32, tag="tq")
        nc.gpsimd.iota(sbi[:].rearrange("p (l s) -> p l s", l=NLG),
                       pattern=[[0, NLG], [W, NS]], base=0, channel_multiplier=0)
        sbase = med.tile([128, GSL], F32, tag="scrB")
        nc.vector.tensor_copy(sbase[:], sbi[:])

        # lvl rows [NS, NL]
        lr_i = sml.tile([NS, NL], I32)
        nc.gpsimd.iota(lr_i[:], pattern=[[1, NL]], base=0, channel_multiplier=0)
        lvl_row = sml.tile([NS, NL], F32)
        nc.vector.tensor_copy(lvl_row[:], lr_i[:])
        nc.vector.tensor_scalar(lvl_row[:], lvl_row[:], 0.1, 0.3, Alu.mult, Alu.add)
        l5f = sml.tile([NS, NL], F32)
        nc.vector.tensor_copy(l5f[:], lr_i[:])

        # ================= loads =================
        vt = big.tile([128, F], F32, tag="bigA")
        nc.sync.dma_start(vt[:].rearrange("p (s x) -> p s x", s=NS),
                          t_in[:, :, :].rearrange("s y x -> y s x"))
        vo = big.tile([128, F], F32, tag="bigO")
        nc.sync.dma_start(vo[:].rearrange("p (s x) -> p s x", s=NS),
                          o_in[:, :, :].rearrange("s y x -> y s x"))

        # ============ minmax O (VE reduces; scheduler runs them early) ======
        mxpO = med.tile([128, NS], F32, name="mxpO", tag="mxpO")
        nc.vector.tensor_reduce(mxpO[:], vo[:].rearrange("p (s x) -> p s x", s=NS), AX.X, Alu.max)
        mnpO = med.tile([128, NS], F32, name="mnpO", tag="mnpO")
        nc.vector.tensor_reduce(mnpO[:], vo[:].rearrange("p (s x) -> p s x", s=NS), AX.X, Alu.min)
        mxbO = med.tile([128, NS], F32, name="mxbO")
        nc.gpsimd.partition_all_reduce(mxbO[:], mxpO[:], channels=128, reduce_op=bass_isa.ReduceOp.max)
        nc.vector.tensor_scalar(mnpO[:], mnpO[:], -1.0, None, Alu.mult)
        mnbO = med.tile([128, NS], F32, name="mnbO")
        nc.gpsimd.partition_all_reduce(mnbO[:], mnpO[:], channels=128, reduce_op=bass_isa.ReduceOp.max)
        rngbO = med.tile([128, NS], F32, name="rngbO")
        nc.vector.tensor_scalar(mnbO[:], mnbO[:], -1.0, None, Alu.mult)
        nc.vector.tensor_tensor(out=rngbO[:], in0=mxbO[:], in1=mnbO[:], op=Alu.subtract)

        # ================= minmax T (VE) + normalize =================
        mxpT = med.tile([128, NS], F32, name="mxpT", tag="mxpT")
        nc.vector.tensor_reduce(mxpT[:], vt[:].rearrange("p (s x) -> p s x", s=NS), AX.X, Alu.max)
        mnpT = med.tile([128, NS], F32, name="mnpT", tag="mnpT")
        nc.vector.tensor_reduce(mnpT[:], vt[:].rearrange("p (s x) -> p s x", s=NS), AX.X, Alu.min)
        mxbT = med.tile([128, NS], F32, name="mxbT")
        nc.gpsimd.partition_all_reduce(mxbT[:], mxpT[:], channels=128, reduce_op=bass_isa.ReduceOp.max)
        nc.vector.tensor_scalar(mnpT[:], mnpT[:], -1.0, None, Alu.mult)
        mnbT = med.tile([128, NS], F32, name="mnbT")
        nc.gpsimd.partition_all_reduce(mnbT[:], mnpT[:], channels=128, reduce_op=bass_isa.ReduceOp.max)
        rngbT = med.tile([128, NS], F32, name="rngbT")
        nc.vector.tensor_scalar(mnbT[:], mnbT[:], -1.0, None, Alu.mult)
        nc.vector.tensor_tensor(out=rngbT[:], in0=mxbT[:], in1=mnbT[:], op=Alu.subtract)

        def normalize(v, mnb, rngb, sfx):
            rngp = med.tile([128, NS], F32, name=f"rngp{sfx}", tag="rngp" + sfx)
            nc.vector.tensor_scalar(rngp[:], rngb[:], float(EPS), None, Alu.add)
            rb = med.tile([128, NS], F32, name=f"rb{sfx}")
            nc.vector.reciprocal(rb[:], rngp[:])
            shift = med.tile([128, NS], F32, name=f"shift{sfx}", tag="shift" + sfx)
            nc.vector.tensor_scalar(shift[:], rngp[:], 0.5, None, Alu.mult)
            nc.vector.tensor_tensor(out=shift[:], in0=shift[:], in1=mnb[:], op=Alu.add)
            nc.vector.tensor_tensor(out=v[:].rearrange("p (s x) -> p s x", s=NS),
                                    in0=v[:].rearrange("p (s x) -> p s x", s=NS),
                                    in1=shift[:].to_broadcast((128, NS, W)), op=Alu.subtract)
            nc.vector.tensor_tensor(out=v[:].rearrange("p (s x) -> p s x", s=NS),
                                    in0=v[:].rearrange("p (s x) -> p s x", s=NS),
                                    in1=rb[:].to_broadcast((128, NS, W)), op=Alu.mult)

        normalize(vt, mnbT, rngbT, "T")
        up = vt

        # ============== fields + moments (target), SA [NS, 72] ==============
        # SA cols: l*6+j for F-moms, 36 + l*6+j for sign-moms
        SA = med.tile([NS, 72], F32)

        HW2 = W // 2
        NGH = NG // 2

        def moments(fA, fB, SAdst, col0):
            psm = ps.tile([128, NS * GP], F32, name=f"psm{col0}{SAdst.name}", tag="psmom")
            fvA = fA[:].rearrange("p (s x) -> p s x", s=NS)
            fvB = fB[:].rearrange("p (s x) -> p s x", s=NS)
            for g in range(NG):
                fv = fvA if g < NGH else fvB
                go = g if g < NGH else g - NGH
                nc.tensor.matmul(psm[:], table[:, g * 128:(g + 1) * 128],
                                 fv[:, :, GP * go:GP * go + GP],
                                 start=(g == 0), stop=(g == NG - 1))
            S96 = med.tile([128, NS * GP], F32, name=f"s96_{col0}{SAdst.name}", tag="s96")
            nc.scalar.activation(S96[:], psm[:], Act.Identity, bias=0.0, scale=1.0)
            pT = ps.tile([NS, 6], F32, name=f"pT{col0}{SAdst.name}", tag="ps6", bufs=1)
            sv = S96[:].rearrange("q (s t) -> q s t", s=NS)
            for t in range(GP):
                nc.tensor.matmul(pT[:], sv[:, :, t], SELS[t][:],
                                 start=(t == 0), stop=(t == GP - 1))
            nc.vector.tensor_copy(SAdst[:, col0 * 6:col0 * 6 + 6], pT[:])

        def mk_field_halves(src_up, pfx, l, lvlp, bias):
            upv = src_up[:].rearrange("p (s x) -> p s x", s=NS)
            fA = fld.tile([128, NS * HW2], BF16, name=f"{pfx}fA{l}", tag="fhalf", bufs=2)
            nc.vector.tensor_scalar(fA[:].rearrange("p (s x) -> p s x", s=NS),
                                    upv[:, :, 0:HW2], lvlp, lvlp, Alu.max, Alu.subtract)
            fB = fld.tile([128, NS * HW2], BF16, name=f"{pfx}fB{l}", tag="fhalf", bufs=2)
            nc.vector.tensor_scalar(fB[:].rearrange("p (s x) -> p s x", s=NS),
                                    upv[:, :, HW2:W], lvlp, lvlp, Alu.max, Alu.subtract)
            gA = fld.tile([128, NS * HW2], BF16, name=f"{pfx}gA{l}", tag="ghalf", bufs=2)
            nc.scalar.activation(gA[:].rearrange("p (s x) -> p s x", s=NS),
                                 upv[:, :, 0:HW2], Act.Sign, bias=bias, scale=1.0)
            gB = fld.tile([128, NS * HW2], BF16, name=f"{pfx}gB{l}", tag="ghalf", bufs=2)
            nc.scalar.activation(gB[:].rearrange("p (s x) -> p s x", s=NS),
                                 upv[:, :, HW2:W], Act.Sign, bias=bias, scale=1.0)
            return fA, fB, gA, gB

        for l in range(NL):
            lvlp = float(LEVELS[l] - np.float32(0.5))
            fA, fB, gA, gB = mk_field_halves(up, "t", l, lvlp, lvl_bias[l][:])
            moments(fA, fB, SA, l)
            moments(gA, gB, SA, 6 + l)

        # ================= prefix scan (after fields on VE) =================
        P = big.tile([128, F + 1], F32, tag="bigP")
        nc.gpsimd.memset(P[:, 0:1], 0.0)
        nc.vector.tensor_tensor_scan(P[:, 1:F + 1], up[:], up[:], 0.0, Alu.add, Alu.bypass)

        # ================= target params (sample layout [NS, NL]) ==========
        def FA(j):
            return SA[:, j:j + 25:6]

        def GA(j):
            return SA[:, 36 + j:36 + j + 25:6]

        Wm = {}
        for j in range(6):
            cb = float(C_B[j])
            mk = med.tile([NS, NL], F32, name=f"mk{j}")
            nc.vector.tensor_scalar(mk[:], GA(j), cb, 0.5, Alu.add, Alu.mult)
            Wj = med.tile([NS, NL], F32, name=f"W{j}")
            nc.vector.tensor_tensor(out=Wj[:], in0=mk[:], in1=lvl_row[:], op=Alu.mult)
            nc.vector.tensor_tensor(out=Wj[:], in0=Wj[:], in1=FA(j), op=Alu.add)
            Wm[j] = Wj

        def fit_params(Wd, tagp, n_l, want_roots):
            def nt(nm):
                return med.tile([NS, n_l], F32, name=tagp + nm)
            m00_ = nt("m00")
            nc.vector.tensor_scalar(m00_[:], Wd[0][:], float(EPS), None, Alu.add)
            im_ = nt("im")
            nc.vector.reciprocal(im_[:], m00_[:])
            cx_ = nt("cx"); cy_ = nt("cy"); tz = nt("tz")
            nc.vector.tensor_tensor(out=cx_[:], in0=Wd[3][:], in1=im_[:], op=Alu.mult)
            nc.vector.tensor_tensor(out=cy_[:], in0=Wd[1][:], in1=im_[:], op=Alu.mult)
            mu20_ = nt("mu20"); mu02_ = nt("mu02"); mu11_ = nt("mu11")
            nc.vector.tensor_tensor(out=mu20_[:], in0=Wd[5][:], in1=im_[:], op=Alu.mult)
            nc.vector.tensor_tensor(out=tz[:], in0=cx_[:], in1=cx_[:], op=Alu.mult)
            nc.vector.tensor_tensor(out=mu20_[:], in0=mu20_[:], in1=tz[:], op=Alu.subtract)
            nc.vector.tensor_tensor(out=mu02_[:], in0=Wd[2][:], in1=im_[:], op=Alu.mult)
            nc.vector.tensor_tensor(out=tz[:], in0=cy_[:], in1=cy_[:], op=Alu.mult)
            nc.vector.tensor_tensor(out=mu02_[:], in0=mu02_[:], in1=tz[:], op=Alu.subtract)
            nc.vector.tensor_tensor(out=mu11_[:], in0=Wd[4][:], in1=im_[:], op=Alu.mult)
            nc.vector.tensor_tensor(out=tz[:], in0=cx_[:], in1=cy_[:], op=Alu.mult)
            nc.vector.tensor_tensor(out=mu11_[:], in0=mu11_[:], in1=tz[:], op=Alu.subtract)
            dmu_ = nt("dmu"); smu_ = nt("smu"); cc_ = nt("cc")
            nc.vector.tensor_tensor(out=dmu_[:], in0=mu20_[:], in1=mu02_[:], op=Alu.subtract)
            nc.vector.tensor_tensor(out=smu_[:], in0=mu20_[:], in1=mu02_[:], op=Alu.add)
            nc.vector.tensor_tensor(out=cc_[:], in0=dmu_[:], in1=dmu_[:], op=Alu.mult)
            nc.vector.tensor_tensor(out=tz[:], in0=mu11_[:], in1=mu11_[:], op=Alu.mult)
            nc.vector.tensor_scalar(tz[:], tz[:], 4.0, None, Alu.mult)
            nc.vector.tensor_tensor(out=cc_[:], in0=cc_[:], in1=tz[:], op=Alu.add)
            com_ = nt("com")
            nc.scalar.sqrt(com_[:], cc_[:])
            gd = nt("gd"); rc = nt("rc")
            nc.vector.tensor_scalar(gd[:], com_[:], 1e-30, None, Alu.max)
            nc.vector.reciprocal(rc[:], gd[:])
            nc.vector.tensor_tensor(out=rc[:], in0=cc_[:], in1=rc[:], op=Alu.mult)
            nc.vector.tensor_tensor(out=com_[:], in0=com_[:], in1=rc[:], op=Alu.add)
            nc.vector.tensor_scalar(com_[:], com_[:], 0.5, None, Alu.mult)
            a2_ = nt("a2"); b2_ = nt("b2")
            nc.vector.tensor_tensor(out=a2_[:], in0=smu_[:], in1=com_[:], op=Alu.add)
            nc.vector.tensor_scalar(a2_[:], a2_[:], 2.0, float(EPS), Alu.mult, Alu.max)
            nc.vector.tensor_tensor(out=b2_[:], in0=smu_[:], in1=com_[:], op=Alu.subtract)
            nc.vector.tensor_scalar(b2_[:], b2_[:], 2.0, float(EPS), Alu.mult, Alu.max)
            a_ = nt("a"); b_ = nt("b")
            nc.scalar.sqrt(a_[:], a2_[:])
            nc.vector.tensor_scalar(gd[:], a_[:], 1e-30, None, Alu.max)
            nc.vector.reciprocal(rc[:], gd[:])
            nc.vector.tensor_tensor(out=rc[:], in0=a2_[:], in1=rc[:], op=Alu.mult)
            nc.vector.tensor_tensor(out=a_[:], in0=a_[:], in1=rc[:], op=Alu.add)
            nc.vector.tensor_scalar(a_[:], a_[:], 0.5, None, Alu.mult)
            nc.scalar.sqrt(b_[:], b2_[:])
            nc.vector.tensor_scalar(gd[:], b_[:], 1e-30, None, Alu.max)
            nc.vector.reciprocal(rc[:], gd[:])
            nc.vector.tensor_tensor(out=rc[:], in0=b2_[:], in1=rc[:], op=Alu.mult)
            nc.vector.tensor_tensor(out=b_[:], in0=b_[:], in1=rc[:], op=Alu.add)
            nc.vector.tensor_scalar(b_[:], b_[:], 0.5, None, Alu.mult)
            cg = nt("cg"); ic = nt("ic")
            nc.vector.tensor_scalar(cg[:], com_[:], 1e-30, None, Alu.max)
            nc.vector.reciprocal(ic[:], cg[:])
            cphi_ = nt("cphi"); sphi_ = nt("sphi")
            nc.vector.tensor_tensor(out=cphi_[:], in0=dmu_[:], in1=ic[:], op=Alu.mult)
            nc.vector.tensor_scalar(cphi_[:], cphi_[:], -1.0, 1.0, Alu.max, Alu.min)
            nc.vector.tensor_tensor(out=sphi_[:], in0=mu11_[:], in1=ic[:], op=Alu.mult)
            cth_ = nt("cth"); sth_ = nt("sth"); sg_ = nt("sg")
            nc.vector.tensor_scalar(cth_[:], cphi_[:], 1.0, 0.5, Alu.add, Alu.mult)
            nc.scalar.sqrt(cth_[:], cth_[:])
            nc.vector.tensor_scalar(sth_[:], cphi_[:], -1.0, None, Alu.mult)
            nc.vector.tensor_scalar(sth_[:], sth_[:], 1.0, 0.5, Alu.add, Alu.mult)
            nc.scalar.sqrt(sth_[:], sth_[:])
            nc.vector.tensor_scalar(sg_[:], sphi_[:], 0.0, None, Alu.is_ge)
            nc.vector.tensor_scalar(sg_[:], sg_[:], 2.0, -1.0, Alu.mult, Alu.add)
            nc.vector.tensor_tensor(out=sth_[:], in0=sth_[:], in1=sg_[:], op=Alu.mult)
            res = dict(cx=cx_, cy=cy_, cth=cth_, sth=sth_, a=a_, b=b_)
            if not want_roots:
                return res
            # roots coefs
            Aa = nt("Aa"); Bb = nt("Bb")
            nc.vector.tensor_scalar(Aa[:], a_[:], float(EPS), None, Alu.add)
            nc.vector.tensor_scalar(Bb[:], b_[:], float(EPS), None, Alu.add)
            iA2 = nt("iA2"); iB2 = nt("iB2")
            nc.vector.tensor_tensor(out=gd[:], in0=Aa[:], in1=Aa[:], op=Alu.mult)
            nc.vector.reciprocal(iA2[:], gd[:])
            nc.vector.tensor_tensor(out=gd[:], in0=Bb[:], in1=Bb[:], op=Alu.mult)
            nc.vector.reciprocal(iB2[:], gd[:])
            c2t = nt("c2t"); s2t = nt("s2t")
            nc.vector.tensor_tensor(out=c2t[:], in0=cth_[:], in1=cth_[:], op=Alu.mult)
            nc.vector.tensor_tensor(out=s2t[:], in0=sth_[:], in1=sth_[:], op=Alu.mult)
            Pq = nt("Pq")
            nc.vector.tensor_tensor(out=Pq[:], in0=c2t[:], in1=iA2[:], op=Alu.mult)
            nc.vector.tensor_tensor(out=tz[:], in0=s2t[:], in1=iB2[:], op=Alu.mult)
            nc.vector.tensor_tensor(out=Pq[:], in0=Pq[:], in1=tz[:], op=Alu.add)
            Rq = nt("Rq")
            nc.vector.tensor_tensor(out=Rq[:], in0=iA2[:], in1=iB2[:], op=Alu.subtract)
            nc.vector.tensor_tensor(out=Rq[:], in0=Rq[:], in1=cth_[:], op=Alu.mult)
            nc.vector.tensor_tensor(out=Rq[:], in0=Rq[:], in1=sth_[:], op=Alu.mult)
            K3 = nt("K3")
            nc.vector.tensor_tensor(out=K3[:], in0=iA2[:], in1=iB2[:], op=Alu.mult)
            iP = nt("iP")
            nc.vector.reciprocal(iP[:], Pq[:])
            K1 = nt("K1")
            nc.vector.tensor_tensor(out=K1[:], in0=Rq[:], in1=iP[:], op=Alu.mult)
            M0 = nt("M0")
            nc.vector.tensor_tensor(out=M0[:], in0=K1[:], in1=cy_[:], op=Alu.mult)
            nc.vector.tensor_tensor(out=M0[:], in0=M0[:], in1=cx_[:], op=Alu.add)
            H2 = nt("H2")
            nc.vector.tensor_scalar(H2[:], K3[:], -1.0, None, Alu.mult)
            H1 = nt("H1")
            nc.vector.tensor_tensor(out=H1[:], in0=K3[:], in1=cy_[:], op=Alu.mult)
            nc.vector.tensor_scalar(H1[:], H1[:], 2.0, None, Alu.mult)
            H0 = nt("H0")
            nc.vector.tensor_tensor(out=tz[:], in0=H1[:], in1=cy_[:], op=Alu.mult)
            nc.vector.tensor_scalar(tz[:], tz[:], 0.5, None, Alu.mult)
            nc.vector.tensor_tensor(out=H0[:], in0=Pq[:], in1=tz[:], op=Alu.subtract)
            res.update(M0=M0, K1=K1, H0=H0, H1=H1, H2=H2, iP=iP)
            return res

        pt = fit_params(Wm, "pt", NL, True)

        # ================= back-broadcast coef rows =================
        BS = med.tile([NS, 30], F32)
        for qi, q in enumerate([pt["M0"], pt["K1"], pt["H0"], pt["H1"], pt["H2"], pt["iP"]]):
            nc.vector.tensor_copy(BS[:, qi * 5:qi * 5 + 5], q[:])
        psb2 = ps1.tile([30, NS], F32, tag="psmisc")
        nc.tensor.transpose(psb2[:], BS[:, :], eye128[0:NS, 0:NS])
        BT = med.tile([30, NS], F32)
        nc.vector.tensor_copy(BT[:], psb2[:])
        bc = {}
        for qi, nm in enumerate(["M0", "K1", "H0", "H1", "H2", "iP"]):
            row = sml.tile([1, SL], F32, name=f"row{nm}")
            nc.sync.dma_start(row[:].rearrange("p (l s) -> p l s", l=NL),
                              BT[qi * 5:qi * 5 + 5, :])
            t128 = med.tile([128, SL], F32, name=f"bc{nm}", tag="bc" + nm)
            nc.gpsimd.partition_broadcast(t128[:], row[:], channels=128)
            bc[nm] = t128

        # ================= roots [128, SL] =================
        arg = med.tile([128, SL], F32)
        nc.vector.tensor_scalar(arg[:], bc["H2"][:], y2v[:], None, Alu.mult)
        tq = med.tile([128, SL], F32, tag="tq")
        nc.vector.tensor_scalar(tq[:], bc["H1"][:], yv[:], None, Alu.mult)
        nc.vector.tensor_tensor(out=arg[:], in0=arg[:], in1=tq[:], op=Alu.add)
        nc.vector.tensor_tensor(out=arg[:], in0=arg[:], in1=bc["H0"][:], op=Alu.add)
        valid = med.tile([128, SL], F32)
        nc.vector.tensor_scalar(valid[:], arg[:], 0.0, None, Alu.is_ge)
        nc.vector.tensor_scalar(arg[:], arg[:], 0.0, None, Alu.max)
        rt = med.tile([128, SL], F32)
        nc.scalar.sqrt(rt[:], arg[:])
        rrec = med.tile([128, SL], F32, tag="tq2")
        nc.vector.tensor_scalar(rrec[:], rt[:], 1e-30, None, Alu.max)
        nc.vector.reciprocal(rrec[:], rrec[:])
        nc.vector.tensor_tensor(out=rrec[:], in0=arg[:], in1=rrec[:], op=Alu.mult)
        nc.vector.tensor_tensor(out=rt[:], in0=rt[:], in1=rrec[:], op=Alu.add)
        nc.vector.tensor_scalar(rt[:], rt[:], 0.5, None, Alu.mult)
        half = rt  # in place: half = rt * iP
        nc.vector.tensor_tensor(out=half[:], in0=rt[:], in1=bc["iP"][:], op=Alu.mult)
        mrow = med.tile([128, SL], F32, tag="scrA")
        nc.vector.tensor_scalar(mrow[:], bc["K1"][:], yv[:], None, Alu.mult)
        nc.vector.tensor_tensor(out=mrow[:], in0=bc["M0"][:], in1=mrow[:], op=Alu.subtract)
        xlo = med.tile([128, SL], F32, tag="tq")
        nc.vector.tensor_tensor(out=xlo[:], in0=mrow[:], in1=half[:], op=Alu.subtract)
        nc.vector.tensor_scalar(xlo[:], xlo[:], 0.0, 127.0, Alu.max, Alu.min)
        xhi = med.tile([128, SL], F32, tag="scrG")
        nc.vector.tensor_tensor(out=xhi[:], in0=mrow[:], in1=half[:], op=Alu.add)
        nc.vector.tensor_scalar(xhi[:], xhi[:], 0.0, 127.0, Alu.max, Alu.min)
        nint = med.tile([128, SL], I32, tag="tq2")
        nc.vector.tensor_copy(nint[:], xhi[:])
        nhi = med.tile([128, SL], F32)
        nc.vector.tensor_copy(nhi[:], nint[:])
        fhi = med.tile([128, SL], F32, tag="tq3")
        nc.vector.tensor_tensor(out=fhi[:], in0=nhi[:], in1=xhi[:], op=Alu.is_gt)
        nc.vector.tensor_tensor(out=nhi[:], in0=nhi[:], in1=fhi[:], op=Alu.subtract)
        nc.vector.tensor_copy(nint[:], xlo[:])
        nlo = med.tile([128, SL], F32)
        nc.vector.tensor_copy(nlo[:], nint[:])
        frac = med.tile([128, SL], F32, tag="tq3")
        nc.vector.tensor_tensor(out=frac[:], in0=xlo[:], in1=nlo[:], op=Alu.is_gt)
        nc.vector.tensor_tensor(out=nlo[:], in0=nlo[:], in1=frac[:], op=Alu.add)
        # nhi1 = nhi + 1 (shared by cnt, gather idx, mask bounds)
        nc.vector.tensor_scalar(nhi[:], nhi[:], 1.0, None, Alu.add)
        nhi1 = nhi
        cnt = med.tile([128, SL], F32)
        nc.vector.tensor_tensor(out=cnt[:], in0=nhi1[:], in1=nlo[:], op=Alu.subtract)
        nc.vector.tensor_scalar(cnt[:], cnt[:], 0.0, None, Alu.max)
        nc.vector.tensor_tensor(out=cnt[:], in0=cnt[:], in1=valid[:], op=Alu.mult)

        # gather idx (levels < NLG): int16 of n*valid + sbase
        gq = med.tile([128, GSL], F32, name="gq", tag="tq2")
        nc.vector.tensor_tensor(out=gq[:], in0=nhi1[:, 0:GSL], in1=valid[:, 0:GSL], op=Alu.mult)
        nc.vector.tensor_tensor(out=gq[:], in0=gq[:], in1=sbase[:], op=Alu.add)
        ih16 = med.tile([128, GSL], I16, tag="ih16")
        nc.vector.tensor_copy(ih16[:], gq[:])
        nc.vector.tensor_tensor(out=gq[:], in0=nlo[:, 0:GSL], in1=valid[:, 0:GSL], op=Alu.mult)
        nc.vector.tensor_tensor(out=gq[:], in0=gq[:], in1=sbase[:], op=Alu.add)
        il16 = med.tile([128, GSL], I16, tag="il16")
        nc.vector.tensor_copy(il16[:], gq[:])

        # mask bounds (levels >= NLG) in bf16: hiM = nhi1*valid - 1 ; loM = nlo
        hiMf = med.tile([128, MSL], F32, name="hiMf", tag="tq3")
        nc.vector.tensor_tensor(out=hiMf[:], in0=nhi1[:, GSL:SL], in1=valid[:, GSL:SL], op=Alu.mult)
        nc.vector.tensor_scalar(hiMf[:], hiMf[:], -1.0, None, Alu.add)
        hiM = med.tile([128, MSL], BF16, name="hiM", tag="bcH1")
        nc.vector.tensor_copy(hiM[:], hiMf[:])
        loM = med.tile([128, MSL], BF16, name="loM", tag="bcH2")
        nc.vector.tensor_copy(loM[:], nlo[:, GSL:SL])

        normalize(vo, mnbO, rngbO, "O")
        upO = vo

        # ================= masks (levels NLG..NL-1) =========================
        # I1row [1, SL] assembled from both paths
        I1row = sml.tile([1, SL], F32)
        XCH = SCH * W  # chunk columns
        for l in range(NLG, NL):
            lm = l - NLG
            for c in range(NCH):
                scol = c * SCH
                mA = fld.tile([128, XCH], BF16, name=f"mA{l}_{c}", tag="mskA")
                nc.vector.tensor_tensor(
                    out=mA[:].rearrange("p (s x) -> p s x", s=SCH),
                    in0=xsx[:].rearrange("p (s x) -> p s x", s=SCH),
                    in1=loM[:, lm * NS + scol:lm * NS + scol + SCH].to_broadcast((128, SCH, W)),
                    op=Alu.is_ge)
                mB = fld.tile([128, XCH], BF16, name=f"mB{l}_{c}", tag="mskB")
                nc.vector.tensor_tensor(
                    out=mB[:].rearrange("p (s x) -> p s x", s=SCH),
                    in0=xsx[:].rearrange("p (s x) -> p s x", s=SCH),
                    in1=hiM[:, lm * NS + scol:lm * NS + scol + SCH].to_broadcast((128, SCH, W)),
                    op=Alu.is_le)
                nc.vector.tensor_tensor(out=mA[:], in0=mA[:], in1=mB[:], op=Alu.mult)
                pc = fld.tile([128, XCH], F32, name=f"pc{l}_{c}", tag="mskP")
                nc.vector.tensor_tensor(out=pc[:], in0=up[:, c * XCH:(c + 1) * XCH],
                                        in1=mA[:], op=Alu.mult)
                # column sums via PE: accumulate over xa -> [1, (s, 32)]
                psm0 = ps1.tile([1, SCH * 32], F32, tag="psm0", bufs=2)
                pcv = pc[:].rearrange("p (s xa xb) -> p s xa xb", s=SCH, xa=4)
                for k in range(4):
                    nc.tensor.matmul(psm0[:], onescol[:], pcv[:, :, k, :],
                                     start=(k == 0), stop=(k == 3))
                S0 = sml.tile([1, SCH * 32], F32, name=f"S0_{l}_{c}", tag="S0row")
                nc.scalar.activation(S0[:], psm0[:], Act.Identity, bias=0.0, scale=1.0)
                nc.vector.tensor_reduce(I1row[:, l * NS + scol:l * NS + scol + SCH],
                                        S0[:].rearrange("p (s b) -> p s b", s=SCH),
                                        AX.X, Alu.add)

        # ======== output-side fields at ALL levels (during window) ========
        SB = med.tile([NS, 72], F32)
        for l in range(NL):
            lvlp = float(LEVELS[l] - np.float32(0.5))
            fA, fB, gA, gB = mk_field_halves(upO, "o", l, lvlp, lvl_bias[l][:])
            moments(fA, fB, SB, l)
            moments(gA, gB, SB, 6 + l)

        def FB(j):
            return SB[:, j:j + 25:6]

        def GB(j):
            return SB[:, 36 + j:36 + j + 25:6]

        WmO = {}
        for j in range(6):
            cb = float(C_B[j])
            mko = med.tile([NS, NL], F32, name=f"mko{j}")
            nc.vector.tensor_scalar(mko[:], GB(j), cb, 0.5, Alu.add, Alu.mult)
            wj = med.tile([NS, NL], F32, name=f"WO{j}")
            nc.vector.tensor_tensor(out=wj[:], in0=mko[:], in1=lvl_row[:], op=Alu.mult)
            nc.vector.tensor_tensor(out=wj[:], in0=wj[:], in1=FB(j), op=Alu.add)
            WmO[j] = wj

        po = fit_params(WmO, "po", NL, False)

        # ================= gathers (levels 0..NLG-1) + extraction ===========
        psI1 = ps1.tile([1, GSL], F32, tag="psI1")
        Ghi = big.tile([128, GSL * 16], F32, name="ghi", tag="bigL")
        nc.gpsimd.ap_gather(Ghi[:], P[:], ih16[:],
                            channels=128, num_elems=F + 1, d=1, num_idxs=GSL * 16)
        gvh = Ghi[:].rearrange("p (sl j) -> p sl j", j=16)
        for j in range(16):
            nc.tensor.matmul(psI1[:], eye16[:, j:j + 1],
                             gvh[:, :, j], start=(j == 0), stop=False)
        Glo = big.tile([128, GSL * 16], F32, name="glo", tag="bigL")
        nc.gpsimd.ap_gather(Glo[:], P[:], il16[:],
                            channels=128, num_elems=F + 1, d=1, num_idxs=GSL * 16)
        gvl = Glo[:].rearrange("p (sl j) -> p sl j", j=16)
        for j in range(16):
            nc.tensor.matmul(psI1[:], eyeneg16[:, j:j + 1],
                             gvl[:, :, j], start=False, stop=(j == 15))
        nc.vector.tensor_copy(I1row[:, 0:GSL], psI1[:])
        psI0 = ps1.tile([1, SL], F32, tag="psI0")
        nc.tensor.matmul(psI0[:], onescol[:], cnt[:], start=True, stop=True)

        # ================= i_tot =================
        btile = med.tile([128, NS + 1], F32)
        nc.vector.tensor_copy(btile[:], P[:, 0:F + 1:W])
        psb = ps1.tile([1, NS + 1], F32, tag="psmisc")
        nc.tensor.matmul(psb[:], onescol[:], btile[:], start=True, stop=True)
        brow = sml.tile([1, NS + 1], F32)
        nc.vector.tensor_copy(brow[:], psb[:])
        itot = sml.tile([1, NS], F32)
        nc.vector.tensor_tensor(out=itot[:], in0=brow[:, 1:NS + 1], in1=brow[:, 0:NS], op=Alu.subtract)
        nc.vector.tensor_scalar(itot[:], itot[:], 0.5 * NPIX, float(EPS), Alu.add, Alu.add)
        itr = sml.tile([1, NS], F32)
        nc.vector.reciprocal(itr[:], itot[:])

        # ================= metric + argmax =================
        I0r = sml.tile([1, SL], F32)
        nc.vector.tensor_copy(I0r[:], psI0[:])
        iin = sml.tile([1, SL], F32)
        nc.vector.tensor_scalar(iin[:], I0r[:], 0.5, None, Alu.mult)
        nc.vector.tensor_tensor(out=iin[:], in0=iin[:], in1=I1row[:], op=Alu.add)
        met = sml.tile([1, SL], F32)
        nc.vector.tensor_tensor(out=met[:].rearrange("p (l s) -> p s l", l=NL),
                                in0=iin[:].rearrange("p (l s) -> p s l", l=NL),
                                in1=itr[:].to_broadcast((1, NS, NL)), op=Alu.mult)
        nc.vector.tensor_scalar(I0r[:], I0r[:], float(1.0 / NPIX), None, Alu.mult)
        nc.vector.tensor_tensor(out=met[:], in0=met[:], in1=I0r[:], op=Alu.subtract)
        mmax = sml.tile([1, NS], F32)
        nc.vector.tensor_reduce(mmax[:], met[:].rearrange("p (l s) -> p s l", l=NL), AX.X, Alu.max)
        lidx_i = sml.tile([1, SL], I32)
        nc.gpsimd.iota(lidx_i[:].rearrange("p (l s) -> p l s", l=NL),
                       pattern=[[1, NL], [0, NS]], base=0, channel_multiplier=0)
        cand = sml.tile([1, SL], F32)
        nc.vector.tensor_copy(cand[:], lidx_i[:])
        eqmax = sml.tile([1, SL], F32)
        nc.vector.tensor_tensor(out=eqmax[:].rearrange("p (l s) -> p s l", l=NL),
                                in0=met[:].rearrange("p (l s) -> p s l", l=NL),
                                in1=mmax[:].to_broadcast((1, NS, NL)), op=Alu.is_lt)
        # cand = l + 99*(met < max)
        nc.vector.tensor_scalar(eqmax[:], eqmax[:], 99.0, None, Alu.mult)
        nc.vector.tensor_tensor(out=cand[:], in0=cand[:], in1=eqmax[:], op=Alu.add)
        bestr = sml.tile([1, NS], F32)
        nc.vector.tensor_reduce(bestr[:], cand[:].rearrange("p (l s) -> p s l", l=NL), AX.X, Alu.min)

        # best as column [NS, 1] via PE transpose
        prb = ps1.tile([NS, 1], F32, tag="psmisc")
        nc.tensor.transpose(prb[:], bestr[:], eye128[0:1, 0:1])
        bestc = med.tile([NS, 1], F32, name="bestc")
        nc.vector.tensor_copy(bestc[:], prb[:])

        # select params at best level (target pt and output po)
        eqm = med.tile([NS, NL], F32, name="eqm")
        nc.vector.tensor_scalar(eqm[:], l5f[:], bestc[:], None, Alu.is_equal)

        def select(src, nm):
            o = med.tile([NS, 1], F32, name="sel" + nm)
            tmp = med.tile([NS, NL], F32, name="selt" + nm, tag="seltmp")
            nc.vector.tensor_tensor(out=tmp[:], in0=src[:], in1=eqm[:], op=Alu.mult)
            nc.vector.tensor_reduce(o[:], tmp[:], AX.X, Alu.add)
            return o

        cxT = select(pt["cx"], "cx"); cyT = select(pt["cy"], "cy")
        cthT = select(pt["cth"], "ct"); sthT = select(pt["sth"], "st")
        aT = select(pt["a"], "a"); bT = select(pt["b"], "b")
        cxO = select(po["cx"], "ocx"); cyO = select(po["cy"], "ocy")
        cthO = select(po["cth"], "oct"); sthO = select(po["sth"], "ost")
        aO = select(po["a"], "oa"); bO = select(po["b"], "ob")

        # ================= sym loss =================
        def col(nm):
            return med.tile([NS, 1], F32, name=nm)

        sc = col("sc")
        nc.vector.tensor_tensor(out=sc[:], in0=aO[:], in1=bO[:], op=Alu.max)
        t1c = col("t1c")
        nc.vector.tensor_tensor(out=t1c[:], in0=aT[:], in1=bT[:], op=Alu.max)
        nc.vector.tensor_tensor(out=sc[:], in0=sc[:], in1=t1c[:], op=Alu.max)
        nc.vector.tensor_scalar(sc[:], sc[:], float(EPS), None, Alu.add)
        isc = col("isc")
        nc.vector.reciprocal(isc[:], sc[:])
        lossc = col("lossc")
        td = col("td")

        def sqdiff_acc(xo, xt, first=False):
            nc.vector.tensor_tensor(out=td[:], in0=xo, in1=xt, op=Alu.subtract)
            nc.vector.tensor_tensor(out=td[:], in0=td[:], in1=isc[:], op=Alu.mult)
            nc.vector.tensor_tensor(out=td[:], in0=td[:], in1=td[:], op=Alu.mult)
            if first:
                nc.vector.tensor_copy(lossc[:], td[:])
            else:
                nc.vector.tensor_tensor(out=lossc[:], in0=lossc[:], in1=td[:], op=Alu.add)

        sqdiff_acc(cxO[:], cxT[:], first=True)
        sqdiff_acc(cyO[:], cyT[:])
        sqdiff_acc(aO[:], aT[:])
        sqdiff_acc(bO[:], bT[:])
        nc.vector.tensor_scalar(lossc[:], lossc[:], 0.5, None, Alu.mult)
        csum = col("csum")
        nc.vector.tensor_tensor(out=csum[:], in0=cthO[:], in1=cthT[:], op=Alu.mult)
        nc.vector.tensor_tensor(out=td[:], in0=sthO[:], in1=sthT[:], op=Alu.mult)
        nc.vector.tensor_tensor(out=csum[:], in0=csum[:], in1=td[:], op=Alu.add)
        nc.vector.tensor_scalar(csum[:], csum[:], -1.0, 1.0, Alu.mult, Alu.add)
        nc.vector.tensor_tensor(out=lossc[:], in0=lossc[:], in1=csum[:], op=Alu.add)

        nc.sync.dma_start(loss_out[:, :], lossc[:])

        DBG("SA", SA[:])
        DBG("met", met[:])
        DBG("bestr", bestr[:])
        DBG("I1row", I1row[:])
        DBG("itr", itr[:])


def build(NS=64, num_devices=1, debug=False):
    nc = bacc.Bacc("TRN2", target_bir_lowering=False, debug=False, num_devices=num_devices)
    with tile.TileContext(nc) as tc:
        emit(nc, tc, NS=NS, debug=debug)
    nc.compile()
    return nc


# ======================================================================
# Host-side entry point: full inputs -> shard across 8 cores -> gather
# ======================================================================
_CACHED = {}


def _get_nc():
    if "nc" not in _CACHED:
        _CACHED["nc"] = build(NS=64, num_devices=8)
    return _CACHED["nc"]


def kernel(output, target):
    from concourse.bass_utils import run_bass_kernel_spmd

    output = np.ascontiguousarray(output, dtype=np.float32)
    target = np.ascontiguousarray(target, dtype=np.float32)
    B = output.shape[0]
    n_cores = 8
    per = B // n_cores
    nc = _get_nc()
    in_maps = []
    for c in range(n_cores):
        sl = slice(c * per, (c + 1) * per)
        in_maps.append({"t": target[sl], "o": output[sl]})
    res = run_bass_kernel_spmd(nc, in_maps, core_ids=list(range(n_cores)))
    losses = np.concatenate([r["loss"].reshape(-1) for r in res.results])
    return np.float32(losses.mean(dtype=np.float64))
